# revision 11
# baseline (speedup 1.0000x reference)
"""Trainium2 Bass kernel for a 12-head attention block (bf16, JIT-woven).

Problem (hardcoded): x [16, 1024, 768] f32, w_qkv [2304, 768], w_proj
[768, 768], b_proj [768].  out = proj(softmax(q k^T / sqrt(64)) v).

Sharding: pure data parallel over batch - 16 batches / 8 cores = 2
batches per core, no collectives.  Host transposes and converts to bf16.

Per-core kernel (all matmuls bf16, fp32 PSUM accumulation):
  - QK^T per head pair (heads 2p / 2p+1 live on partitions 0-63 / 64-127
    of qt/kt): the two heads' matmuls are emitted adjacently so they run
    concurrently in different PE row-quadrants (tile_position (0,0)/(64,0)).
  - exp on ACT, one [128,1024] instr per (head, jc).  ACT paces the
    attention, so QKV/proj matmuls of the other batch are woven in as
    "filler" groups through a dedicated 1-bank PSUM pool.
  - PV: V ones-augmented (M=65, softmax denominator rides output row 64),
    contraction split into two 64-row halves emitted adjacently for
    row-quadrant concurrency, accumulating into the same PSUM bank.
    Head h0's PV runs inline with its pair's jc loop; h1's PV is deferred
    one pair (pt buffered in SBUF, hf-major through one PSUM bank) so the
    pair pipeline never stalls ACT.
  - normalize: 1/l on DVE, partition-broadcast on GPSIMD, mul on DVE.

PSUM: s_pool 2x[128,1024] (4 banks) + o_pool 3x[65,512] (3) +
f_pool 1x[128,512] (1) = 8 banks.
"""

import ml_dtypes
import numpy as np
from contextlib import ExitStack

import concourse.bass as bass
import concourse.mybir as mybir
import concourse.tile as tile
from concourse import bacc
from concourse import bass_utils

F32 = mybir.dt.float32
BF16 = mybir.dt.bfloat16
EXP = mybir.ActivationFunctionType.Exp

B, N, C = 16, 1024, 768
H, D = 12, 64
E = 3 * C
NCORES = 8
BL = B // NCORES          # batches per core
T = BL * N                # tokens per core
KC = C // 128             # feature chunks of 128
JC = N // 128             # token chunks of 128
NP = H // 2               # head pairs
SCALE = float(D) ** -0.5

_CACHE = {}


def _build(ctx, tc):
    nc = tc.nc
    mm = nc.tensor.matmul
    dram = ctx.enter_context(tc.tile_pool(name="dram", bufs=1, space="DRAM"))
    xT_d = dram.tile([KC, BL, 128, N], BF16, kind="ExternalInput", name="xTb", uniquify=False)
    wqkv_d = dram.tile([KC, 128, E], BF16, kind="ExternalInput", name="wqkvb", uniquify=False)
    wproj_d = dram.tile([KC, 128, C], BF16, kind="ExternalInput", name="wprojb", uniquify=False)
    bproj_d = dram.tile([C, 1], F32, kind="ExternalInput", name="bproj", uniquify=False)
    outT_d = dram.tile([KC, BL, 128, N], F32, kind="ExternalOutput", name="outTb", uniquify=False)

    consts = ctx.enter_context(tc.tile_pool(name="consts", bufs=1))
    wqk_pool = ctx.enter_context(tc.tile_pool(name="wqk", bufs=KC))
    wv_pool = ctx.enter_context(tc.tile_pool(name="wv", bufs=KC))
    wp_pool = ctx.enter_context(tc.tile_pool(name="wproj", bufs=KC))
    xt_pool = ctx.enter_context(tc.tile_pool(name="xtp", bufs=8))
    ot_pool = ctx.enter_context(tc.tile_pool(name="otp", bufs=8))
    qk_pool = ctx.enter_context(tc.tile_pool(name="qkp", bufs=4))
    va_pool = ctx.enter_context(tc.tile_pool(name="vap", bufs=2 * JC))
    pt0_pool = ctx.enter_context(tc.tile_pool(name="pt0p", bufs=3))
    pt1_pool = ctx.enter_context(tc.tile_pool(name="pt1p", bufs=16))
    l_pool = ctx.enter_context(tc.tile_pool(name="lp", bufs=3))
    lb_pool = ctx.enter_context(tc.tile_pool(name="lbp", bufs=3))
    ob_pool = ctx.enter_context(tc.tile_pool(name="obp", bufs=2))

    s_pool = ctx.enter_context(tc.tile_pool(name="sps", bufs=2, space="PSUM"))
    o0_pool = ctx.enter_context(tc.tile_pool(name="o0ps", bufs=2, space="PSUM"))
    o1_pool = ctx.enter_context(tc.tile_pool(name="o1ps", bufs=1, space="PSUM"))
    f_pool = ctx.enter_context(tc.tile_pool(name="fps", bufs=1, space="PSUM"))

    vones = consts.tile([128, H, 1], BF16)
    nc.vector.memset(vones, 1.0)
    bias_sb = consts.tile([128, KC], F32)
    nc.sync.dma_start(out=bias_sb, in_=bproj_d[:, 0].rearrange("(k p) -> p k", p=128))

    # ---- weight + batch-0 x preloads ----
    xt = {}   # (b, kc) -> [128, N] bf16
    for kc in range(KC):
        t = xt_pool.tile([128, N], BF16, name=f"xt0_{kc}", tag="xt")
        nc.sync.dma_start(out=t, in_=xT_d[kc, 0])
        xt[(0, kc)] = t
    wqk_t = []
    for kc in range(KC):
        t = wqk_pool.tile([128, 2 * KC, 128], BF16, name=f"wqk{kc}", tag="wqk")
        nc.sync.dma_start(out=t.rearrange("p a b -> p (a b)"), in_=wqkv_d[kc, :, 0:2 * C])
        wqk_t.append(t)
    wqk = {(j, kc): wqk_t[kc][:, j, :] for j in range(2 * KC) for kc in range(KC)}
    wv = []
    for kc in range(KC):
        t = wv_pool.tile([128, KC, 128], BF16, name=f"wv{kc}", tag="wv")
        nc.sync.dma_start(out=t.rearrange("p a b -> p (a b)"), in_=wqkv_d[kc, :, 2 * C:3 * C])
        wv.append(t)
    wp = {}

    qt = {}  # b -> [128, KC, N] bf16 (head-major feature layout)
    kt = {}
    for b in range(BL):
        qt[b] = qk_pool.tile([128, KC, N], BF16, name=f"qtt{b}", tag="qk")
        kt[b] = qk_pool.tile([128, KC, N], BF16, name=f"ktt{b}", tag="qk")
    va = {}  # (b, jc) -> [128, H, D+1] bf16, ones-augmented
    ot = {}  # (b, p) -> [128, N] bf16

    # ---------------- filler work units ----------------
    def fill_qk(b, which, mt, hf):
        dest = qt[b] if which == 0 else kt[b]
        f = f_pool.tile([128, 512], F32, name=f"fqk{b}_{which}_{mt}_{hf}", tag="f")
        for kc in range(KC):
            mm(f, wqk[(which * KC + mt, kc)], xt[(b, kc)][:, hf * 512:(hf + 1) * 512],
               start=(kc == 0), stop=(kc == KC - 1))
        nc.vector.tensor_copy(out=dest[:, mt, hf * 512:(hf + 1) * 512], in_=f)

    def fill_v(b, jc, half):
        if half == 0:
            vat = va_pool.tile([128, H, D + 1], BF16, name=f"va{b}_{jc}", tag="va")
            va[(b, jc)] = vat
            nc.vector.tensor_copy(out=vat[:, :, D:D + 1], in_=vones)
        vat = va[(b, jc)]
        f = f_pool.tile([128, 512], F32, name=f"fv{b}_{jc}_{half}", tag="f")
        w = 512 if half == 0 else 256
        for kc in range(KC):
            xs = xt[(b, kc)][:, jc * 128:(jc + 1) * 128]
            wvf = wv[kc].rearrange("p a b -> p (a b)")
            mm(f[:, 0:w], xs, wvf[:, half * 512:half * 512 + w],
               start=(kc == 0), stop=(kc == KC - 1))
        h0 = half * 8
        nh = 8 if half == 0 else 4
        nc.vector.tensor_copy(
            out=vat[:, h0:h0 + nh, 0:D],
            in_=f[:, 0:w].rearrange("p (h d) -> p h d", h=nh),
        )

    def fill_proj(b, oc, hf):
        f = f_pool.tile([128, 512], F32, name=f"fp{b}_{oc}_{hf}", tag="f")
        for kc in range(KC):
            mm(f, wp[(kc, oc)], ot[(b, kc)][:, hf * 512:(hf + 1) * 512],
               start=(kc == 0), stop=(kc == KC - 1))
        ob = ob_pool.tile([128, 512], F32, name=f"ob{b}_{oc}_{hf}", tag="ob")
        nc.vector.tensor_scalar_add(out=ob, in0=f, scalar1=bias_sb[:, oc:oc + 1])
        nc.sync.dma_start(out=outT_d[oc, b, :, hf * 512:(hf + 1) * 512], in_=ob)

    def fill_xdma(b):
        for kc in range(KC):
            t = xt_pool.tile([128, N], BF16, name=f"xt{b}_{kc}", tag="xt")
            nc.sync.dma_start(out=t, in_=xT_d[kc, b])
            xt[(b, kc)] = t

    def fill_wp():
        for kc in range(KC):
            t = wp_pool.tile([128, KC, 128], BF16, name=f"wpk{kc}", tag="wp")
            nc.sync.dma_start(out=t.rearrange("p a b -> p (a b)"), in_=wproj_d[kc])
            for oc in range(KC):
                wp[(kc, oc)] = t[:, oc, :]

    fillers = []

    def drain(n):
        for _ in range(min(n, len(fillers))):
            fillers.pop(0)()

    # ---------------- startup: qk(b0, mt0/mt1) via s_pool ----------------
    def startup_qk(b, which, mt):
        dest = qt[b] if which == 0 else kt[b]
        s = s_pool.tile([128, N], F32, name=f"sqk{b}_{which}_{mt}", tag="s")
        for hf in range(2):
            for kc in range(KC):
                mm(s[:, hf * 512:(hf + 1) * 512],
                   wqk[(which * KC + mt, kc)],
                   xt[(b, kc)][:, hf * 512:(hf + 1) * 512],
                   start=(kc == 0), stop=(kc == KC - 1))
        nc.vector.tensor_copy(out=dest[:, mt, :], in_=s)

    for mt in (0, 1):
        for which in (0, 1):
            startup_qk(0, which, mt)
    fill_wp()

    # filler queue for batch-0 attention
    for jc in range(JC):
        for half in (0, 1):
            fillers.append(lambda b=0, jc=jc, half=half: fill_v(b, jc, half))
    fillers.append(lambda: fill_xdma(1))
    for mt in range(2, KC):
        for which in (0, 1):
            for hf in (0, 1):
                fillers.append(lambda w=which, mt=mt, hf=hf: fill_qk(0, w, mt, hf))
    for mt in range(KC):
        for which in (0, 1):
            for hf in (0, 1):
                fillers.append(lambda w=which, mt=mt, hf=hf: fill_qk(1, w, mt, hf))
    for jc in range(JC):
        for half in (0, 1):
            fillers.append(lambda jc=jc, half=half: fill_v(1, jc, half))

    # ---------------- normalization helpers ----------------
    def normalize_h0(b, p, o_hf):
        # o_hf: two [65,512] psum tiles (hf halves); head at ot rows 0:64
        l_sb = l_pool.tile([1, N], F32, name=f"l0_{b}_{p}", tag="l")
        nc.vector.tensor_copy(out=l_sb[:, 0:512], in_=o_hf[0][D:D + 1, :])
        nc.vector.tensor_copy(out=l_sb[:, 512:N], in_=o_hf[1][D:D + 1, :])
        nc.vector.reciprocal_approx_fast(out=l_sb, in_=l_sb)
        lb = lb_pool.tile([D, N], F32, name=f"lb0_{b}_{p}", tag="lb")
        nc.gpsimd.partition_broadcast(lb, l_sb, channels=D)
        for hf in range(2):
            nc.vector.tensor_mul(
                out=ot[(b, p)][0:D, hf * 512:(hf + 1) * 512],
                in0=o_hf[hf][0:D, :], in1=lb[:, hf * 512:(hf + 1) * 512])

    def normalize_h1_hf(b, p, o_t, hf):
        # one hf half of deferred head h1; head at ot rows 64:128
        l_sb = l_pool.tile([1, 512], F32, name=f"l1_{b}_{p}_{hf}", tag="l")
        nc.vector.tensor_copy(out=l_sb, in_=o_t[D:D + 1, :])
        nc.vector.reciprocal_approx_fast(out=l_sb, in_=l_sb)
        lb = lb_pool.tile([D, 512], F32, name=f"lb1_{b}_{p}_{hf}", tag="lb")
        nc.gpsimd.partition_broadcast(lb, l_sb, channels=D)
        nc.vector.tensor_mul(
            out=ot[(b, p)][64:128, hf * 512:(hf + 1) * 512],
            in0=o_t[0:D, :], in1=lb)

    # ---------------- pair-pipelined attention ----------------
    # global pair index g: pairs (b, p) = (g // NP, g % NP); pair g's h1-PV
    # runs during pair g+1's jc loop (hf-major, one psum bank at a time).
    NG = BL * NP
    prev_state = None  # (b, p, pts1)

    for g in range(NG + 1):
        cur = (g // NP, g % NP) if g < NG else None
        if cur is not None:
            b, p = cur
            ot[(b, p)] = ot_pool.tile([128, N], BF16, name=f"ot{b}_{p}", tag="ot")
            o0 = [o0_pool.tile([D + 1, 512], F32, name=f"o0_{b}_{p}_{hf}", tag="o0")
                  for hf in range(2)]
            pts0, pts1 = {}, {}
        if prev_state is not None:
            pb, pp, ppts1 = prev_state
            o1 = None
        if g == NP + 1:  # after b0 h1-tail is emitted: queue proj(b0)
            for oc in range(KC):
                for hf in (0, 1):
                    fillers.append(lambda oc=oc, hf=hf: fill_proj(0, oc, hf))

        ndrain = 2 if (g % NP) in (0, 4, 5) else 1
        for step in range(JC):
            if cur is not None:
                b, p = cur
                s0 = s_pool.tile([128, N], F32, name=f"s0_{b}_{p}_{step}", tag="s")
                s1 = s_pool.tile([128, N], F32, name=f"s1_{b}_{p}_{step}", tag="s")
                jc = step
                for hf in range(2):
                    mm(s0[:, hf * 512:(hf + 1) * 512],
                       kt[b][0:64, p, jc * 128:(jc + 1) * 128],
                       qt[b][0:64, p, hf * 512:(hf + 1) * 512])
                    mm(s1[:, hf * 512:(hf + 1) * 512],
                       kt[b][64:128, p, jc * 128:(jc + 1) * 128],
                       qt[b][64:128, p, hf * 512:(hf + 1) * 512])
                pt0 = pt0_pool.tile([128, N], BF16, name=f"pt0_{b}_{p}_{jc}", tag="pt0")
                pt1 = pt1_pool.tile([128, N], BF16, name=f"pt1_{b}_{p}_{jc}", tag="pt1")
                nc.scalar.activation(out=pt0, in_=s0, func=EXP, scale=SCALE)
                nc.scalar.activation(out=pt1, in_=s1, func=EXP, scale=SCALE)
                pts0[jc] = pt0
                pts1[jc] = pt1
                if g == 0:
                    drain(2)  # V(b0) JIT ahead of this jc's PV
                # h0 PV: jc-major
                for hf in range(2):
                    mm(o0[hf][0:D + 1, :],
                       va[(b, jc)][:, 2 * p, :],
                       pts0[jc][:, hf * 512:(hf + 1) * 512],
                       start=(jc == 0), stop=(jc == JC - 1))
            # deferred h1 PV of previous pair: hf-major, 2 jc-chunks per step
            if prev_state is not None:
                hf, q = divmod(step, 4)
                if q == 0:
                    o1 = o1_pool.tile([D + 1, 512], F32,
                                     name=f"o1_{pb}_{pp}_{hf}", tag="o1")
                for jj in (2 * q, 2 * q + 1):
                    mm(o1[0:D + 1, :],
                       va[(pb, jj)][:, 2 * pp + 1, :],
                       ppts1[jj][:, hf * 512:(hf + 1) * 512],
                       start=(jj == 0), stop=(jj == JC - 1))
                if q == 3:
                    normalize_h1_hf(pb, pp, o1, hf)
            if g > 0:
                drain(ndrain)
        if cur is not None:
            normalize_h0(b, p, o0)
            prev_state = (b, p, pts1)
        drain(2)

    # ---------------- tail: proj(b1) ----------------
    for oc in range(KC):
        for hf in range(2):
            fill_proj(BL - 1, oc, hf)
    drain(len(fillers))


def get_nc():
    if "nc" not in _CACHE:
        nc = bacc.Bacc(None, target_bir_lowering=False, debug=False)
        with tile.TileContext(nc) as tc:
            with ExitStack() as ctx:
                _build(ctx, tc)
        nc.compile()
        _CACHE["nc"] = nc
    return _CACHE["nc"]


def make_in_maps(x, w_qkv, w_proj, b_proj):
    bf16 = ml_dtypes.bfloat16
    x = np.asarray(x, dtype=np.float32).astype(bf16)
    w_qkv = np.asarray(w_qkv, dtype=np.float32).astype(bf16)
    w_proj = np.asarray(w_proj, dtype=np.float32).astype(bf16)
    wqkvb = np.ascontiguousarray(w_qkv.T.reshape(KC, 128, E))
    wprojb = np.ascontiguousarray(w_proj.T.reshape(KC, 128, C))
    bp = np.ascontiguousarray(np.asarray(b_proj, dtype=np.float32).reshape(C, 1))
    in_maps = []
    for c in range(NCORES):
        xT = np.ascontiguousarray(x[c * BL:(c + 1) * BL].reshape(T, C).T)
        xb = np.ascontiguousarray(xT.reshape(KC, 128, BL, N).transpose(0, 2, 1, 3))
        in_maps.append({"xTb": xb, "wqkvb": wqkvb, "wprojb": wprojb, "bproj": bp})
    return in_maps


def assemble_out(results):
    outs = []
    for c in range(NCORES):
        ob = results[c]["outTb"]
        oT = ob.transpose(0, 2, 1, 3).reshape(C, T)
        outs.append(np.ascontiguousarray(oT.T).reshape(BL, N, C))
    return np.concatenate(outs, axis=0).astype(np.float32)


def kernel(x, w_qkv, w_proj, b_proj):
    nc = get_nc()
    in_maps = make_in_maps(x, w_qkv, w_proj, b_proj)
    res = bass_utils.run_bass_kernel_spmd(nc, in_maps, core_ids=list(range(NCORES)))
    return assemble_out(res.results)


# revision 14
# speedup vs baseline: 1.0207x; 1.0207x over previous
"""Trainium2 Bass kernel for a 12-head attention block.

Problem (hardcoded): x [16, 1024, 768] f32, w_qkv [2304, 768], w_proj
[768, 768], b_proj [768].  out = proj(softmax(q k^T / sqrt(64)) v).

Sharding: pure data parallel over batch — 16 batches / 8 cores = 2
batches per core, no collectives.  All layout transposes happen on the
host: each core receives x^T [768, 2048] and produces out^T [768, 2048].

Per-core kernel (all matmuls in float32r = relaxed-precision fp32,
1 cycle/row when the moving dim >= 256):
  A) qkv projection into transposed layouts:
       qT/kT [768(head-major), 1024] per batch, V natural [j, d] per
       j-chunk augmented with a ones column (for the softmax denominator).
  B) per head: S^T[j,i] = K^T.T @ Q^T chunks -> exp on ACT (scale folded)
       -> O^T_aug[65, i] = V_aug.T @ P^T accumulated over j-chunks in PSUM.
       Row 64 of O^T_aug is the softmax denominator l[i].
  C) normalize: r = 1/l on DVE, broadcast r across 64 partitions via a
       0-stride SBUF->SBUF DMA, multiply on DVE.
  D) proj: out^T = w_proj^T.T @ O^T + b_proj, DMA back to DRAM.
"""

import ml_dtypes
import numpy as np
from contextlib import ExitStack

import concourse.bass as bass
import concourse.mybir as mybir
import concourse.tile as tile
from concourse import bacc
from concourse import bass_utils

F32 = mybir.dt.float32
BF16 = mybir.dt.bfloat16
EXP = mybir.ActivationFunctionType.Exp

B, N, C = 16, 1024, 768
H, D = 12, 64
E = 3 * C
NCORES = 8
BL = B // NCORES          # batches per core
T = BL * N                # tokens per core
KC = C // 128             # feature chunks of 128
JC = N // 128             # token chunks of 128
SCALE = float(D) ** -0.5

_CACHE = {}


def _mm(nc, out, lhsT, rhs, **kw):
    nc.tensor.matmul(out, lhsT=lhsT, rhs=rhs, **kw)


def _build(ctx, tc):
    nc = tc.nc
    dram = ctx.enter_context(tc.tile_pool(name="dram", bufs=1, space="DRAM"))
    # x^T blocked: [kc, b, 128, N] so each per-batch chunk is one contiguous slab
    xT_d = dram.tile([KC, BL, 128, N], BF16, kind="ExternalInput", name="xTb", uniquify=False)
    # w_qkv^T as per-kc slabs [kc, 128, 2304]: DMA rows are 6KB/3KB contiguous
    wqkv_d = dram.tile([KC, 128, E], BF16, kind="ExternalInput", name="wqkvb", uniquify=False)
    # w_proj^T per-kc slabs [kc, 128, 768]
    wproj_d = dram.tile([KC, 128, C], BF16, kind="ExternalInput", name="wprojb", uniquify=False)
    bproj_d = dram.tile([C, 1], F32, kind="ExternalInput", name="bproj", uniquify=False)
    # out^T blocked: [oc, b, 128, N]
    outT_d = dram.tile([KC, BL, 128, N], F32, kind="ExternalOutput", name="outTb", uniquify=False)

    consts = ctx.enter_context(tc.tile_pool(name="consts", bufs=1))
    wp_pool = ctx.enter_context(tc.tile_pool(name="wproj", bufs=KC))
    wqk_pool = ctx.enter_context(tc.tile_pool(name="wqk", bufs=KC))
    wv_pool = ctx.enter_context(tc.tile_pool(name="wv", bufs=KC))
    xo_pool = ctx.enter_context(tc.tile_pool(name="xo", bufs=KC))
    qk_pool = ctx.enter_context(tc.tile_pool(name="qkpool", bufs=2))
    v_pool = ctx.enter_context(tc.tile_pool(name="vpool", bufs=JC))
    p_pool = ctx.enter_context(tc.tile_pool(name="ppool", bufs=2))
    sm_pool = ctx.enter_context(tc.tile_pool(name="small", bufs=3))
    ps_pool = ctx.enter_context(tc.tile_pool(name="psbig", bufs=2, space="PSUM"))
    po_pool = ctx.enter_context(tc.tile_pool(name="psO", bufs=2, space="PSUM"))

    vones_f32 = consts.tile([128, H, 1], F32)
    nc.vector.memset(vones_f32, 1.0)
    bias_sb = consts.tile([128, KC], F32)
    nc.sync.dma_start(
        out=bias_sb, in_=bproj_d[:, 0].rearrange("(k p) -> p k", p=128)
    )

    # batch-0 x first: the very first matmuls wait on these, so their DMAs
    # must be at the head of the queues, ahead of the bulk weight preload.
    xt0 = []
    for kc in range(KC):
        xtc = xo_pool.tile([128, N], BF16, name=f"xt0_{kc}", tag="xo")
        nc.sync.dma_start(out=xtc, in_=xT_d[kc, 0])
        xt0.append(xtc)
    # q/k weights resident: per-kc [128, 12, 128] tile, ONE wide DMA each
    wqk_t = []
    for kc in range(KC):
        t = wqk_pool.tile([128, 2 * KC, 128], BF16, name=f"wqk{kc}", tag="wqk")
        nc.sync.dma_start(out=t.rearrange("p a b -> p (a b)"), in_=wqkv_d[kc, :, 0:2 * C])
        wqk_t.append(t)
    wqk = {(j, kc): wqk_t[kc][:, j, :] for j in range(2 * KC) for kc in range(KC)}
    # v weights per-kc [128, 6, 128] (V matmul rhs needs a contiguous 768 span)
    wv = []
    for kc in range(KC):
        wvt = wv_pool.tile([128, KC, 128], BF16, name=f"wv{kc}", tag="wv")
        nc.sync.dma_start(out=wvt.rearrange("p a b -> p (a b)"), in_=wqkv_d[kc, :, 2 * C:3 * C])
        wv.append(wvt)
    wp = {}

    for b in range(BL):
        # ---- load x^T for this batch ----
        if b == 0:
            xt = xt0
        else:
            xt = []
            for kc in range(KC):
                xtc = xo_pool.tile([128, N], BF16, name=f"xt{b}_{kc}", tag="xo")
                nc.sync.dma_start(out=xtc, in_=xT_d[kc, b])
                xt.append(xtc)

        # ---- A: q^T and k^T, head-major feature layout ----
        qt = qk_pool.tile([128, KC, N], BF16, name=f"qt{b}", tag="qk")
        kt = qk_pool.tile([128, KC, N], BF16, name=f"kt{b}", tag="qk")
        for which, dest in ((0, qt), (1, kt)):
            for mt in range(KC):
                ps = ps_pool.tile([128, N], F32, name=f"psqk{b}_{which}_{mt}", tag="big")
                for kc in range(KC):
                    w = wqk[(which * KC + mt, kc)]
                    for hf in range(2):
                        _mm(nc, ps[:, hf * 512:(hf + 1) * 512],
                            w, xt[kc][:, hf * 512:(hf + 1) * 512],
                            start=(kc == 0), stop=(kc == KC - 1))
                nc.vector.tensor_copy(out=dest[:, mt, :], in_=ps)

        # ---- A: V in natural [j, d] layout, ones-augmented ----
        va = []
        for jc in range(JC):
            vps = ps_pool.tile([128, C], F32, name=f"vps{b}_{jc}", tag="big")
            for kc in range(KC):
                xs = xt[kc][:, jc * 128:(jc + 1) * 128]
                wvf = wv[kc].rearrange("p a b -> p (a b)")
                _mm(nc, vps[:, 0:512], xs, wvf[:, 0:512],
                    start=(kc == 0), stop=(kc == KC - 1))
                _mm(nc, vps[:, 512:C], xs, wvf[:, 512:C],
                    start=(kc == 0), stop=(kc == KC - 1))
            vat = v_pool.tile([128, H, D + 1], BF16, name=f"va{b}_{jc}", tag="va")
            nc.vector.tensor_copy(
                out=vat[:, :, 0:D], in_=vps.rearrange("p (h d) -> p h d", h=H)
            )
            nc.vector.tensor_copy(out=vat[:, :, D:D + 1], in_=vones_f32)
            va.append(vat)

        if b == 0:
            # defer w_proj loads to here so they don't crowd startup DMA
            for kc in range(KC):
                t = wp_pool.tile([128, KC, 128], BF16, name=f"wpk{kc}", tag="wp")
                nc.sync.dma_start(out=t.rearrange("p a b -> p (a b)"), in_=wproj_d[kc])
                for oc in range(KC):
                    wp[(kc, oc)] = t[:, oc, :]

        # ---- B: attention, one head at a time, software-pipelined so the
        # PE stream is QKT(0) QKT(1) PV(0) QKT(2) PV(1) ... (no in-order
        # stall on the exp) and ACT runs back-to-back exps.
        ot = [xo_pool.tile([128, N], BF16, name=f"ot{b}_{m}", tag="xo") for m in range(KC)]
        for h in range(H):
            mt, off = h // 2, (h % 2) * D
            o_ps = po_pool.tile([D + 1, N], F32, name=f"ops{b}_{h}", tag="ops")

            def qkt(jc):
                s = ps_pool.tile([128, N], F32, name=f"sps{b}_{h}_{jc}", tag="big")
                for hf in range(2):
                    _mm(nc, s[:, hf * 512:(hf + 1) * 512],
                        kt[off:off + D, mt, jc * 128:(jc + 1) * 128],
                        qt[off:off + D, mt, hf * 512:(hf + 1) * 512])
                return s

            s = qkt(0)
            for jc in range(JC):
                pt = p_pool.tile([128, N], BF16, name=f"pt{b}_{h}_{jc}", tag="pt")
                nc.scalar.activation(out=pt, in_=s, func=EXP, scale=SCALE)
                if jc + 1 < JC:
                    s = qkt(jc + 1)
                for hf in range(2):
                    _mm(nc, o_ps[:, hf * 512:(hf + 1) * 512],
                        va[jc][:, h, :], pt[:, hf * 512:(hf + 1) * 512],
                        start=(jc == 0), stop=(jc == JC - 1))
            # normalize: rows 0..63 divided by l (= row 64), broadcast on GPSIMD
            l_sb = sm_pool.tile([1, N], F32, name=f"l{b}_{h}", tag="sm")
            nc.vector.tensor_copy(out=l_sb, in_=o_ps[D:D + 1, :])
            nc.vector.reciprocal_approx_fast(out=l_sb, in_=l_sb)
            lb = sm_pool.tile([D, N], F32, name=f"lb{b}_{h}", tag="sm")
            nc.gpsimd.partition_broadcast(lb, l_sb, channels=D)
            nc.vector.tensor_mul(
                out=ot[mt][off:off + D, :], in0=o_ps[0:D, :], in1=lb
            )

        # ---- C: output projection + bias ----
        for oc in range(KC):
            pps = ps_pool.tile([128, N], F32, name=f"pps{b}_{oc}", tag="big")
            for kc in range(KC):
                for hf in range(2):
                    _mm(nc, pps[:, hf * 512:(hf + 1) * 512],
                        wp[(kc, oc)],
                        ot[kc][:, hf * 512:(hf + 1) * 512],
                        start=(kc == 0), stop=(kc == KC - 1))
            ob = sm_pool.tile([128, N], F32, name=f"ob{b}_{oc}", tag="sm")
            nc.vector.tensor_scalar_add(out=ob, in0=pps, scalar1=bias_sb[:, oc:oc + 1])
            nc.sync.dma_start(out=outT_d[oc, b], in_=ob)


def get_nc():
    if "nc" not in _CACHE:
        nc = bacc.Bacc(None, target_bir_lowering=False, debug=False)
        with tile.TileContext(nc) as tc:
            with ExitStack() as ctx:
                _build(ctx, tc)
        nc.compile()
        _CACHE["nc"] = nc
    return _CACHE["nc"]


def make_in_maps(x, w_qkv, w_proj, b_proj):
    bf16 = ml_dtypes.bfloat16
    x = np.asarray(x, dtype=np.float32).astype(bf16)
    w_qkv = np.asarray(w_qkv, dtype=np.float32).astype(bf16)
    w_proj = np.asarray(w_proj, dtype=np.float32).astype(bf16)
    # w_qkv^T [c, e] -> per-kc slabs [kc, 128, 2304]
    wqkvb = np.ascontiguousarray(w_qkv.T.reshape(KC, 128, E))
    # w_proj^T [c, o] -> per-kc slabs [kc, 128, 768]
    wprojb = np.ascontiguousarray(w_proj.T.reshape(KC, 128, C))
    bp = np.ascontiguousarray(b_proj.astype(np.float32).reshape(C, 1))
    in_maps = []
    for c in range(NCORES):
        # x^T [c, t] -> blocks [kc, b, 128, N]
        xT = x[c * BL:(c + 1) * BL].reshape(T, C).T  # [768, 2048]
        xb = np.ascontiguousarray(
            xT.reshape(KC, 128, BL, N).transpose(0, 2, 1, 3)
        )
        in_maps.append({"xTb": xb, "wqkvb": wqkvb, "wprojb": wprojb, "bproj": bp})
    return in_maps


def assemble_out(results):
    outs = []
    for c in range(NCORES):
        ob = results[c]["outTb"]  # [oc, b, 128, N]
        oT = ob.transpose(0, 2, 1, 3).reshape(C, T)
        outs.append(np.ascontiguousarray(oT.T).reshape(BL, N, C))
    return np.concatenate(outs, axis=0).astype(np.float32)


def kernel(x, w_qkv, w_proj, b_proj):
    nc = get_nc()
    in_maps = make_in_maps(x, w_qkv, w_proj, b_proj)
    res = bass_utils.run_bass_kernel_spmd(nc, in_maps, core_ids=list(range(NCORES)))
    return assemble_out(res.results)

